# revision 10
# baseline (speedup 1.0000x reference)
"""MoE routing kernel for Trainium2: softmax over 256 experts + top-8 per token.

Full input: gating_output [131072, 256] f32. Output: (topk_weights f32,
topk_indices int32), both [131072, 8] — matching jax.lax.top_k semantics
(values descending, ties broken by lowest index first).

Strategy: shard tokens row-wise across 8 NeuronCores (16384 tokens each; the
computation is row-local so no communication). Per core, token = p*128 + tt
(partition-major): partition p owns 128 consecutive tokens, processed in
chunks of T subtiles (T consecutive token rows per partition, so each chunk's
input DMA is 128 descriptors of T KiB contiguous). A short-prologue chunk
schedule lets the compute engines start early.

Engine split per chunk:
  DVE : T x InstMax (top-8 raw logits, descending), then T x InstMaxIndex
        (indices; duplicates get ascending distinct indices — matches
        jax.lax.top_k tie rules), plus a tiny reciprocal. This is the
        bottleneck engine: ~721 ns per subtile is the ISA-model floor
        (two full 256-element scans; the input has exact-duplicate and
        <1.5e-5 near-tie rows at the top-8 boundary, so no approximate /
        compressed selection scheme is exact — both scans are required).
  ACT : ONE fused Exp over the whole chunk [128, T*256] (no accumulator),
        plus Exp on the [128, T*8] top-8 logits. Softmax max-subtraction is
        skipped: |x| <= ~5.5 keeps exp well inside f32 range, and softmax is
        shift-invariant.
  Pool: per-token softmax denominators via a log2 tree of strided adds
        (256 -> 128 -> ... -> 1 per token), then the final weights multiply
        exp(top8) * (1/sums). Moving the denominator off ACT removes the
        per-token ACTIVATE + ACCUMULATOR_READ pair (~105us of ACT time)
        that co-bottlenecked with DVE.

Top-k results accumulate in persistent SBUF buffers and flush to DRAM in
quarter-core batches (4 KiB-contiguous runs per partition) so output DMA is
a few large-descriptor transfers instead of thousands of 256 B ones.

Top-8 selection runs on raw logits (softmax is monotone, so same selection),
which avoids f32 ties introduced by exp rounding.
"""

import numpy as np

TOKENS = 131072
EXPERTS = 256
K = 8
N_CORES = 8
TOK_PER_CORE = TOKENS // N_CORES  # 16384
P = 128
TT = TOK_PER_CORE // P  # 128 token rows per partition

# Subtile counts per chunk: short prologue so the first DMA lands fast and
# compute engines spin up early; steady-state 8-subtile (1 MiB) chunks
# (measured tighter DVE stream than 16-subtile chunks: ~1us vs ~3us of
# slack over the intrinsic max8/find_index8 cadence).
CHUNKS = [1, 3, 4] + [8] * 15
assert sum(CHUNKS) == TT

# Flush the persistent output buffers after these many accumulated subtiles.
# Denominator reduction, reciprocal, and the weights multiply are batched at
# the same granularity (one "group" = 32 subtiles) to amortize the ~550ns
# fixed cost of every Pool-engine instruction.
FLUSH_AT = (32, 64, 96, TT)
GROUP = 32

_PROGRAM_CACHE = {}


def _build_program():
    import concourse.tile as tile
    from concourse import bacc, mybir

    f32 = mybir.dt.float32
    u32 = mybir.dt.uint32
    Exp = mybir.ActivationFunctionType.Exp
    Ln = mybir.ActivationFunctionType.Ln

    nc = bacc.Bacc("TRN2", debug=False, num_devices=N_CORES)

    g_dram = nc.dram_tensor(
        "gating", [TOK_PER_CORE, EXPERTS], f32, kind="ExternalInput"
    ).ap()
    w_dram = nc.dram_tensor(
        "weights", [TOK_PER_CORE, K], f32, kind="ExternalOutput"
    ).ap()
    i_dram = nc.dram_tensor(
        "indices", [TOK_PER_CORE, K], u32, kind="ExternalOutput"
    ).ap()

    # token = p*TT + tt: partition-major views
    g_v = g_dram.rearrange("(p tt) e -> p tt e", p=P)  # [128, 128, 256]
    w_v = w_dram.rearrange("(p tt) k -> p tt k", p=P)  # [128, 128, 8]
    i_v = i_dram.rearrange("(p tt) k -> p tt k", p=P)

    with tile.TileContext(nc) as tc:
        with (
            tc.tile_pool(name="gin", bufs=5) as gin_pool,
            tc.tile_pool(name="expbuf", bufs=2) as exp_pool,
            tc.tile_pool(name="outs", bufs=3) as out_pool,
            tc.tile_pool(name="persist", bufs=1) as persist_pool,
        ):
            # persistent per-core result buffers (8 KiB/partition total)
            wbuf = persist_pool.tile([P, TT, K], f32, name="wbuf")
            ibuf = persist_pool.tile([P, TT, K], u32, name="ibuf")

            # self-managed zero bias for the Exp activations: a float bias
            # would become a const AP whose TENSOR_LOAD delays the sync
            # sequencer's first input DMA by ~1us; a Pool-engine memset is
            # off that critical path.
            zbias = persist_pool.tile([P, 1], f32, name="zbias")
            nc.gpsimd.memset(zbias, 0.0)

            # persistent top-8 raw logits, written per chunk (DVE max8) and
            # consumed per flush group by the weights path
            vbuf = persist_pool.tile([P, TT, K], f32, name="vbuf")

            ct = 0
            flushed = 0
            fi = 0
            et_group = None
            group_base = 0
            for ci, T in enumerate(CHUNKS):
                if et_group is None:
                    # exp buffer covering a whole 32-subtile flush group
                    et_group = exp_pool.tile(
                        [P, GROUP, EXPERTS], f32, name=f"etg{fi}", tag="etg"
                    )
                    group_base = ct
                gt = gin_pool.tile([P, T * EXPERTS], f32, name=f"gt{ci}", tag="gt")
                nc.sync.dma_start(out=gt, in_=g_v[:, ct : ct + T, :])
                gt3 = gt.rearrange("p (t e) -> p t e", t=T)

                for t in range(T):
                    nc.vector.max(out=vbuf[:, ct + t, :], in_=gt3[:, t, :])
                for t in range(T):
                    nc.vector.max_index(
                        out=ibuf[:, ct + t, :],
                        in_max=vbuf[:, ct + t, :],
                        in_values=gt3[:, t, :],
                    )

                # One fused Exp over the whole chunk (ACT) into the group
                # buffer; per-token denominators are reduced once per group.
                go = ct - group_base
                nc.scalar.activation(
                    out=et_group[:, go : go + T, :], in_=gt3, func=Exp, bias=zbias
                )

                ct += T
                if fi < len(FLUSH_AT) and ct >= FLUSH_AT[fi]:
                    g0 = group_base
                    gn = ct - g0
                    assert gn == GROUP
                    # log2 tree of strided adds on Pool: 256 -> 1 per token.
                    # Level 1 folds the exp buffer onto its own low half
                    # (pure elementwise, in-place-safe); later levels use a
                    # small scratch.
                    nc.gpsimd.tensor_tensor(
                        out=et_group[:, :, 0:128],
                        in0=et_group[:, :, 0:128],
                        in1=et_group[:, :, 128:256],
                        op=mybir.AluOpType.add,
                    )
                    sc = exp_pool.tile([P, gn, 127], f32, name=f"sc{fi}", tag="sc")
                    nc.gpsimd.tensor_tensor(
                        out=sc[:, :, 0:64],
                        in0=et_group[:, :, 0:64],
                        in1=et_group[:, :, 64:128],
                        op=mybir.AluOpType.add,
                    )
                    off = 0
                    w = 32
                    while w >= 1:
                        nc.gpsimd.tensor_tensor(
                            out=sc[:, :, off + 2 * w : off + 3 * w],
                            in0=sc[:, :, off : off + w],
                            in1=sc[:, :, off + w : off + 2 * w],
                            op=mybir.AluOpType.add,
                        )
                        off += 2 * w
                        w //= 2
                    sums = sc[:, :, 126:127].rearrange("p t one -> p (t one)")

                    # weights = exp(v - ln(denominator)) — keeps reciprocal
                    # and the normalize multiply OFF the DVE queue entirely.
                    lnd = out_pool.tile([P, gn], f32, name=f"lnd{fi}", tag="lnd")
                    nc.scalar.activation(out=lnd, in_=sums, func=Ln, bias=zbias)
                    evv = out_pool.tile([P, gn, K], f32, name=f"evv{fi}", tag="evv")
                    nc.gpsimd.tensor_tensor(
                        out=evv,
                        in0=vbuf[:, g0:ct, :],
                        in1=lnd.rearrange("p (t one) -> p t one", one=1).to_broadcast(
                            [P, gn, K]
                        ),
                        op=mybir.AluOpType.subtract,
                    )
                    nc.scalar.activation(
                        out=wbuf[:, g0:ct, :], in_=evv, func=Exp, bias=zbias
                    )

                    nc.sync.dma_start(
                        out=w_v[:, flushed:ct, :], in_=wbuf[:, flushed:ct, :]
                    )
                    nc.sync.dma_start(
                        out=i_v[:, flushed:ct, :], in_=ibuf[:, flushed:ct, :]
                    )
                    flushed = ct
                    fi += 1
                    et_group = None

    nc.compile()
    return nc


def kernel(**inputs) -> tuple:
    from concourse.bass_utils import run_bass_kernel_spmd

    gating = np.ascontiguousarray(np.asarray(inputs["gating_output"], dtype=np.float32))
    topk = int(np.asarray(inputs.get("topk", K)))
    assert topk == K, f"kernel hardcodes top-{K}, got topk={topk}"
    assert gating.shape == (TOKENS, EXPERTS), gating.shape

    if "nc" not in _PROGRAM_CACHE:
        _PROGRAM_CACHE["nc"] = _build_program()
    nc = _PROGRAM_CACHE["nc"]

    shards = gating.reshape(N_CORES, TOK_PER_CORE, EXPERTS)
    in_maps = [{"gating": shards[c]} for c in range(N_CORES)]
    res = run_bass_kernel_spmd(nc, in_maps, core_ids=list(range(N_CORES)))
    _PROGRAM_CACHE["last_results"] = res

    weights = np.concatenate([r["weights"] for r in res.results], axis=0)
    indices = np.concatenate([r["indices"] for r in res.results], axis=0)
    return weights.astype(np.float32, copy=False), indices.astype(np.int32, copy=False)



# revision 12
# speedup vs baseline: 1.0067x; 1.0067x over previous
"""MoE routing kernel for Trainium2: softmax over 256 experts + top-8 per token.

Full input: gating_output [131072, 256] f32. Output: (topk_weights f32,
topk_indices int32), both [131072, 8] — matching jax.lax.top_k semantics
(values descending, ties broken by lowest index first).

Strategy: shard tokens row-wise across 8 NeuronCores (16384 tokens each; the
computation is row-local so no communication). Per core, token = p*128 + tt
(partition-major): partition p owns 128 consecutive tokens, processed in
chunks of T subtiles (T consecutive token rows per partition, so each chunk's
input DMA is 128 descriptors of T KiB contiguous). A short-prologue chunk
schedule lets the compute engines start early.

Engine split per chunk:
  DVE : T x InstMax (top-8 raw logits, descending), then T x InstMaxIndex
        (indices; duplicates get ascending distinct indices — matches
        jax.lax.top_k tie rules), plus a tiny reciprocal. This is the
        bottleneck engine: ~721 ns per subtile is the ISA-model floor
        (two full 256-element scans; the input has exact-duplicate and
        <1.5e-5 near-tie rows at the top-8 boundary, so no approximate /
        compressed selection scheme is exact — both scans are required).
  ACT : ONE fused Exp over the whole chunk [128, T*256] (no accumulator),
        plus Exp on the [128, T*8] top-8 logits. Softmax max-subtraction is
        skipped: |x| <= ~5.5 keeps exp well inside f32 range, and softmax is
        shift-invariant.
  Pool: per-token softmax denominators via a log2 tree of strided adds
        (256 -> 128 -> ... -> 1 per token), then the final weights multiply
        exp(top8) * (1/sums). Moving the denominator off ACT removes the
        per-token ACTIVATE + ACCUMULATOR_READ pair (~105us of ACT time)
        that co-bottlenecked with DVE.

Top-k results accumulate in persistent SBUF buffers and flush to DRAM in
quarter-core batches (4 KiB-contiguous runs per partition) so output DMA is
a few large-descriptor transfers instead of thousands of 256 B ones.

Top-8 selection runs on raw logits (softmax is monotone, so same selection),
which avoids f32 ties introduced by exp rounding.
"""

import numpy as np

TOKENS = 131072
EXPERTS = 256
K = 8
N_CORES = 8
TOK_PER_CORE = TOKENS // N_CORES  # 16384
P = 128
TT = TOK_PER_CORE // P  # 128 token rows per partition

# Subtile counts per chunk: short prologue so the first DMA lands fast and
# compute engines spin up early; steady-state 8-subtile (1 MiB) chunks
# (measured tighter DVE stream than 16-subtile chunks: ~1us vs ~3us of
# slack over the intrinsic max8/find_index8 cadence).
CHUNKS = [1, 1, 2, 4, 8, 8, 8] + [8] * 12
assert sum(CHUNKS) == TT

# Flush the persistent output buffers after these many accumulated subtiles.
# Denominator reduction, reciprocal, and the weights multiply are batched at
# the same granularity (one "group" = 32 subtiles) to amortize the ~550ns
# fixed cost of every Pool-engine instruction.
FLUSH_AT = (32, 64, 96, TT)
GROUP = 32

_PROGRAM_CACHE = {}


def _build_program():
    import concourse.tile as tile
    from concourse import bacc, mybir

    f32 = mybir.dt.float32
    u32 = mybir.dt.uint32
    Exp = mybir.ActivationFunctionType.Exp
    Ln = mybir.ActivationFunctionType.Ln

    nc = bacc.Bacc("TRN2", debug=False, num_devices=N_CORES)

    g_dram = nc.dram_tensor(
        "gating", [TOK_PER_CORE, EXPERTS], f32, kind="ExternalInput"
    ).ap()
    w_dram = nc.dram_tensor(
        "weights", [TOK_PER_CORE, K], f32, kind="ExternalOutput"
    ).ap()
    i_dram = nc.dram_tensor(
        "indices", [TOK_PER_CORE, K], u32, kind="ExternalOutput"
    ).ap()

    # token = p*TT + tt: partition-major views
    g_v = g_dram.rearrange("(p tt) e -> p tt e", p=P)  # [128, 128, 256]
    w_v = w_dram.rearrange("(p tt) k -> p tt k", p=P)  # [128, 128, 8]
    i_v = i_dram.rearrange("(p tt) k -> p tt k", p=P)

    with tile.TileContext(nc) as tc:
        with (
            tc.tile_pool(name="gin", bufs=5) as gin_pool,
            tc.tile_pool(name="expbuf", bufs=2) as exp_pool,
            tc.tile_pool(name="outs", bufs=3) as out_pool,
            tc.tile_pool(name="persist", bufs=1) as persist_pool,
        ):
            # persistent per-core result buffers (8 KiB/partition total)
            wbuf = persist_pool.tile([P, TT, K], f32, name="wbuf")
            ibuf = persist_pool.tile([P, TT, K], u32, name="ibuf")

            # self-managed zero bias for the Exp activations: a float bias
            # would become a const AP whose TENSOR_LOAD delays the sync
            # sequencer's first input DMA by ~1us; a Pool-engine memset is
            # off that critical path.
            zbias = persist_pool.tile([P, 1], f32, name="zbias")
            nc.gpsimd.memset(zbias, 0.0)

            # persistent top-8 raw logits, written per chunk (DVE max8) and
            # consumed per flush group by the weights path
            vbuf = persist_pool.tile([P, TT, K], f32, name="vbuf")

            def finalize(fi_, g0, g1, et_g):
                """Weights path for subtiles [g0, g1): Pool add-tree for the
                softmax denominators, then w = exp(v - ln(sum)), then flush.
                Emitted a few chunks AFTER the group completes so the Pool/ACT
                round-trips never stall the in-order engine queues mid-stream.
                """
                gn = g1 - g0
                # log2 tree of strided adds on Pool: 256 -> 1 per token.
                # Level 1 folds the exp buffer onto its own low half (pure
                # elementwise, in-place-safe); later levels use a scratch.
                nc.gpsimd.tensor_tensor(
                    out=et_g[:, :, 0:128],
                    in0=et_g[:, :, 0:128],
                    in1=et_g[:, :, 128:256],
                    op=mybir.AluOpType.add,
                )
                sc = exp_pool.tile([P, gn, 127], f32, name=f"sc{fi_}", tag="sc")
                nc.gpsimd.tensor_tensor(
                    out=sc[:, :, 0:64],
                    in0=et_g[:, :, 0:64],
                    in1=et_g[:, :, 64:128],
                    op=mybir.AluOpType.add,
                )
                off = 0
                w = 32
                while w >= 1:
                    nc.gpsimd.tensor_tensor(
                        out=sc[:, :, off + 2 * w : off + 3 * w],
                        in0=sc[:, :, off : off + w],
                        in1=sc[:, :, off + w : off + 2 * w],
                        op=mybir.AluOpType.add,
                    )
                    off += 2 * w
                    w //= 2
                sums = sc[:, :, 126:127].rearrange("p t one -> p (t one)")

                # weights = exp(v - ln(denominator)) — keeps reciprocal and
                # the normalize multiply OFF the DVE queue entirely.
                lnd = out_pool.tile([P, gn], f32, name=f"lnd{fi_}", tag="lnd")
                nc.scalar.activation(out=lnd, in_=sums, func=Ln, bias=zbias)
                evv = out_pool.tile([P, gn, K], f32, name=f"evv{fi_}", tag="evv")
                nc.gpsimd.tensor_tensor(
                    out=evv,
                    in0=vbuf[:, g0:g1, :],
                    in1=lnd.rearrange("p (t one) -> p t one", one=1).to_broadcast(
                        [P, gn, K]
                    ),
                    op=mybir.AluOpType.subtract,
                )
                nc.scalar.activation(
                    out=wbuf[:, g0:g1, :], in_=evv, func=Exp, bias=zbias
                )
                nc.sync.dma_start(out=w_v[:, g0:g1, :], in_=wbuf[:, g0:g1, :])
                nc.sync.dma_start(out=i_v[:, g0:g1, :], in_=ibuf[:, g0:g1, :])

            ct = 0
            fi = 0
            et_group = None
            group_base = 0
            pending = None
            pend_end = 0
            for ci, T in enumerate(CHUNKS):
                if et_group is None:
                    # exp buffer covering a whole 32-subtile flush group
                    et_group = exp_pool.tile(
                        [P, GROUP, EXPERTS], f32, name=f"etg{fi}", tag="etg"
                    )
                    group_base = ct
                gt = gin_pool.tile([P, T * EXPERTS], f32, name=f"gt{ci}", tag="gt")
                nc.sync.dma_start(out=gt, in_=g_v[:, ct : ct + T, :])
                gt3 = gt.rearrange("p (t e) -> p t e", t=T)

                for t in range(T):
                    nc.vector.max(out=vbuf[:, ct + t, :], in_=gt3[:, t, :])
                for t in range(T):
                    nc.vector.max_index(
                        out=ibuf[:, ct + t, :],
                        in_max=vbuf[:, ct + t, :],
                        in_values=gt3[:, t, :],
                    )

                # One fused Exp over the whole chunk (ACT) into the group
                # buffer; per-token denominators are reduced once per group.
                go = ct - group_base
                nc.scalar.activation(
                    out=et_group[:, go : go + T, :], in_=gt3, func=Exp, bias=zbias
                )

                ct += T
                if pending is not None and ct - pend_end >= 20:
                    finalize(*pending)
                    pending = None
                if fi < len(FLUSH_AT) and ct >= FLUSH_AT[fi]:
                    pending = (fi, group_base, ct, et_group)
                    pend_end = ct
                    fi += 1
                    et_group = None
            if pending is not None:
                finalize(*pending)

    nc.compile()
    return nc


def kernel(**inputs) -> tuple:
    from concourse.bass_utils import run_bass_kernel_spmd

    gating = np.ascontiguousarray(np.asarray(inputs["gating_output"], dtype=np.float32))
    topk = int(np.asarray(inputs.get("topk", K)))
    assert topk == K, f"kernel hardcodes top-{K}, got topk={topk}"
    assert gating.shape == (TOKENS, EXPERTS), gating.shape

    if "nc" not in _PROGRAM_CACHE:
        _PROGRAM_CACHE["nc"] = _build_program()
    nc = _PROGRAM_CACHE["nc"]

    shards = gating.reshape(N_CORES, TOK_PER_CORE, EXPERTS)
    in_maps = [{"gating": shards[c]} for c in range(N_CORES)]
    res = run_bass_kernel_spmd(nc, in_maps, core_ids=list(range(N_CORES)))
    _PROGRAM_CACHE["last_results"] = res

    weights = np.concatenate([r["weights"] for r in res.results], axis=0)
    indices = np.concatenate([r["indices"] for r in res.results], axis=0)
    return weights.astype(np.float32, copy=False), indices.astype(np.int32, copy=False)



# revision 15
# speedup vs baseline: 1.1524x; 1.1447x over previous
"""MoE routing kernel for Trainium2: softmax over 256 experts + top-8 per token.

Full input: gating_output [131072, 256] f32. Output: (topk_weights f32,
topk_indices int32), both [131072, 8] — matching jax.lax.top_k semantics
(values descending, ties broken by lowest index first).

Strategy: shard tokens row-wise across 8 NeuronCores (16384 tokens each; the
computation is row-local so no communication). Per core, token = p*128 + tt
(partition-major): partition p owns 128 consecutive tokens, processed in
chunks of T token rows per partition (T<=4), so each chunk's input DMA is
128 descriptors of T KiB contiguous.

Engine split — DVE is the bottleneck and runs NOTHING but the top-k:
  DVE : per subtile [128, 256]: InstMax (top-8 raw logits, descending) then
        InstMaxIndex (indices; duplicates get ascending distinct indices —
        matches jax.lax.top_k tie rules). Two full 256-element scans per
        subtile is the ISA floor: the input has exact-duplicate and <1.5e-5
        near-tie rows at the top-8 boundary, so no approximate or compressed
        selection scheme is exact. ~723ns/subtile * 128 subtiles ~= 93us.
  PE  : per subtile, two f32 transposes of the raw logits into PSUM
        ([tok, e] -> [e, tok]), then one ones-matmul (one-hot stationary
        selecting PSUM row s) that accumulates this subtile's per-token
        exp-sums into a persistent [128, 2, 128] PSUM tile across all 128
        subtiles. This computes the softmax denominators entirely on the
        otherwise-idle tensor engine.
  ACT : ONE fused Exp per chunk reading the PSUM transpose, writing bf16 to
        SBUF for the matmul (denominator terms only — 0.2% rounding noise,
        well under the 2e-2 tolerance; top-8 weights are recomputed in f32).
        Softmax max-subtraction is skipped: |x| <= 5.5 keeps exp well inside
        f32 range, and softmax is shift-invariant.
  Pool: tiny end-chain ops only. (Pool tensor ops measure ~2.2ns/elem +
        ~550ns/instruction — too slow for any bulk work.)

End-chain (runs ~20us before DVE finishes, thanks to DMA prefetch): copy the
denominator PSUM tile to SBUF, PE-transpose it back to token-major, add the
two expert-half sums, then weights = exp(v - ln(sum)) — Ln+Exp on ACT, no
reciprocal or multiply ever touches the DVE queue. Weights flush in one DMA;
indices flush in 4 rolling group DMAs so only a tiny one remains at the end.
"""

import numpy as np

TOKENS = 131072
EXPERTS = 256
K = 8
N_CORES = 8
TOK_PER_CORE = TOKENS // N_CORES  # 16384
P = 128
TT = TOK_PER_CORE // P  # 128 token rows per partition
HC = 4  # subtiles per chunk (transpose/exp granularity)

# Short prologue so the first DMA lands fast and DVE spins up early.
CHUNKS = [1, 1, 2] + [4] * 31
assert sum(CHUNKS) == TT

# Rolling index-output flushes (weights flush once at the end).
IFLUSH_AT = (32, 64, 96, TT)

_PROGRAM_CACHE = {}


def _build_program():
    import concourse.tile as tile
    from concourse import bacc, masks, mybir

    f32 = mybir.dt.float32
    bf16 = mybir.dt.bfloat16
    u32 = mybir.dt.uint32
    Exp = mybir.ActivationFunctionType.Exp
    Ln = mybir.ActivationFunctionType.Ln

    nc = bacc.Bacc("TRN2", debug=False, num_devices=N_CORES)

    g_dram = nc.dram_tensor(
        "gating", [TOK_PER_CORE, EXPERTS], f32, kind="ExternalInput"
    ).ap()
    w_dram = nc.dram_tensor(
        "weights", [TOK_PER_CORE, K], f32, kind="ExternalOutput"
    ).ap()
    i_dram = nc.dram_tensor(
        "indices", [TOK_PER_CORE, K], u32, kind="ExternalOutput"
    ).ap()

    # token = p*TT + tt: partition-major views
    g_v = g_dram.rearrange("(p tt) e -> p tt e", p=P)  # [128, 128, 256]
    w_v = w_dram.rearrange("(p tt) k -> p tt k", p=P)  # [128, 128, 8]
    i_v = i_dram.rearrange("(p tt) k -> p tt k", p=P)

    with tile.TileContext(nc) as tc:
        with (
            tc.tile_pool(name="gin", bufs=6) as gin_pool,
            tc.tile_pool(name="expbuf", bufs=3) as exp_pool,
            tc.tile_pool(name="outs", bufs=3) as out_pool,
            tc.tile_pool(name="persist", bufs=1) as persist_pool,
            tc.tile_pool(name="psum", bufs=2, space="PSUM") as psum_pool,
            tc.tile_pool(name="psump", bufs=1, space="PSUM") as psump_pool,
        ):
            # persistent per-core result buffers
            vbuf = persist_pool.tile([P, TT, K], f32, name="vbuf")
            ibuf = persist_pool.tile([P, TT, K], u32, name="ibuf")
            wbuf = persist_pool.tile([P, TT, K], f32, name="wbuf")

            # constants: zero bias, f32 identity (PE transpose moving
            # operand), and the sliding one-hot for denominator row select
            zbias = persist_pool.tile([P, 1], f32, name="zbias")
            nc.gpsimd.memset(zbias, 0.0)
            ident = persist_pool.tile([P, P], f32, name="ident")
            masks.make_identity(nc, ident[:])
            oneh = persist_pool.tile([P, 2 * P], bf16, name="oneh")
            nc.gpsimd.memset(oneh, 0.0)
            nc.gpsimd.memset(oneh[:, P - 1 : P], 1.0)

            # persistent PSUM accumulator: dT[s, tok] = sum over all 256
            # experts of exp(logit) for token (tok, subtile s); both expert
            # halves accumulate into the same target
            dT = psump_pool.tile([P, P], f32, name="dT")

            pend_mm = None  # deferred (etT, base_subtile, T) for PE matmuls

            def emit_mms(etT_, s0, T_, last):
                for t in range(T_):
                    s = s0 + t
                    for h in range(2):
                        nc.tensor.matmul(
                            dT,
                            oneh[:, P - 1 - s : 2 * P - 1 - s],
                            etT_[:, t, h, :],
                            start=(s == 0 and h == 0),
                            stop=(last and t == T_ - 1 and h == 1),
                            skip_group_check=True,
                        )

            ct = 0
            ifi = 0
            for ci, T in enumerate(CHUNKS):
                gt = gin_pool.tile([P, T * EXPERTS], f32, name=f"gt{ci}", tag="gt")
                nc.sync.dma_start(out=gt, in_=g_v[:, ct : ct + T, :])
                gt3 = gt.rearrange("p (t e) -> p t e", t=T)

                for t in range(T):
                    nc.vector.max(out=vbuf[:, ct + t, :], in_=gt3[:, t, :])
                for t in range(T):
                    nc.vector.max_index(
                        out=ibuf[:, ct + t, :],
                        in_max=vbuf[:, ct + t, :],
                        in_values=gt3[:, t, :],
                    )

                # PE: transpose this chunk's raw logits into PSUM
                gtT = psum_pool.tile([P, HC, 2, P], f32, name=f"gtT{ci}", tag="gtT")
                for t in range(T):
                    for h in range(2):
                        nc.tensor.transpose(
                            gtT[:, t, h, :],
                            gt3[:, t, h * P : (h + 1) * P],
                            ident,
                        )
                # PE: previous chunk's denominator matmuls (deferred one
                # chunk so the PE queue never waits on ACT mid-stream)
                if pend_mm is not None:
                    emit_mms(*pend_mm, last=False)

                # ACT: fused Exp of the transposed chunk -> bf16 SBUF
                etT = exp_pool.tile([P, T, 2, P], bf16, name=f"etT{ci}", tag="etT")
                nc.scalar.activation(
                    out=etT, in_=gtT[:, :T, :, :], func=Exp, bias=zbias
                )
                pend_mm = (etT, ct, T)

                ct += T
                if ifi < len(IFLUSH_AT) and ct >= IFLUSH_AT[ifi]:
                    lo = IFLUSH_AT[ifi - 1] if ifi else 0
                    nc.sync.dma_start(out=i_v[:, lo:ct, :], in_=ibuf[:, lo:ct, :])
                    ifi += 1

            emit_mms(*pend_mm, last=True)

            # ---- end-chain: denominators -> weights (overlaps DVE tail) ----
            dTs = out_pool.tile([P, P], f32, name="dTs")
            nc.scalar.copy(out=dTs, in_=dT)
            dback = psum_pool.tile([P, P], f32, name="dback", tag="gtT")
            nc.tensor.transpose(dback, dTs, ident)
            # ln of D[p, s] (token (p, s)) straight out of PSUM on ACT
            lnd = out_pool.tile([P, TT], f32, name="lnd")
            nc.scalar.activation(out=lnd, in_=dback, func=Ln, bias=zbias)
            evv = out_pool.tile([P, TT, K], f32, name="evv")
            nc.gpsimd.tensor_tensor(
                out=evv,
                in0=vbuf,
                in1=lnd.rearrange("p (t one) -> p t one", one=1).to_broadcast(
                    [P, TT, K]
                ),
                op=mybir.AluOpType.subtract,
            )
            nc.scalar.activation(out=wbuf, in_=evv, func=Exp, bias=zbias)
            nc.sync.dma_start(out=w_v, in_=wbuf)

    nc.compile()
    return nc


def kernel(**inputs) -> tuple:
    from concourse.bass_utils import run_bass_kernel_spmd

    gating = np.ascontiguousarray(np.asarray(inputs["gating_output"], dtype=np.float32))
    topk = int(np.asarray(inputs.get("topk", K)))
    assert topk == K, f"kernel hardcodes top-{K}, got topk={topk}"
    assert gating.shape == (TOKENS, EXPERTS), gating.shape

    if "nc" not in _PROGRAM_CACHE:
        _PROGRAM_CACHE["nc"] = _build_program()
    nc = _PROGRAM_CACHE["nc"]

    shards = gating.reshape(N_CORES, TOK_PER_CORE, EXPERTS)
    in_maps = [{"gating": shards[c]} for c in range(N_CORES)]
    res = run_bass_kernel_spmd(nc, in_maps, core_ids=list(range(N_CORES)))
    _PROGRAM_CACHE["last_results"] = res

    weights = np.concatenate([r["weights"] for r in res.results], axis=0)
    indices = np.concatenate([r["indices"] for r in res.results], axis=0)
    return weights.astype(np.float32, copy=False), indices.astype(np.int32, copy=False)


# revision 19
# speedup vs baseline: 1.1745x; 1.0193x over previous
"""MoE routing kernel for Trainium2: softmax over 256 experts + top-8 per token.

Full input: gating_output [131072, 256] f32. Output: (topk_weights f32,
topk_indices int32), both [131072, 8] — matching jax.lax.top_k semantics
(values descending, ties broken by lowest index first).

Strategy: shard tokens row-wise across 8 NeuronCores (16384 tokens each; the
computation is row-local so no communication). Per core, token = p*128 + tt
(partition-major): partition p owns 128 consecutive tokens, processed in
chunks of T token rows per partition (T<=4), so each chunk's input DMA is
128 descriptors of T KiB contiguous.

Engine split — DVE is the bottleneck and runs NOTHING but the top-k:
  DVE : per subtile [128, 256]: InstMax (top-8 raw logits, descending) then
        InstMaxIndex (indices; duplicates get ascending distinct indices —
        matches jax.lax.top_k tie rules). Two full 256-element scans per
        subtile is the ISA floor: the input has exact-duplicate and <1.5e-5
        near-tie rows at the top-8 boundary, so no approximate or compressed
        selection scheme is exact. ~723ns/subtile * 128 subtiles ~= 93us.
  ACT : ONE fused Exp per chunk (f32 SBUF -> bf16 SBUF). The bf16 rounding
        only touches denominator terms (~0.1% on the sum, vs 2e-2 tol);
        top-8 weights are recomputed in f32 at the end. Max-subtraction is
        skipped: |x| <= 5.5 keeps exp well in f32 range; softmax is
        shift-invariant.
  PE  : per subtile, two bf16 transposes of exp into PSUM ([tok,e]->[e,tok]),
        then per chunk TWO ones-matmuls (sliding chunk-one-hot stationary)
        that accumulate per-token exp-sums into a persistent [32, 1024] PSUM
        tile — row c holds chunk c's 1024 token denominators. Denominators
        thus cost the otherwise-idle tensor engine ~2 instructions/subtile.
  DMA : copies each chunk's transposed exp PSUM -> SBUF (matmul rhs must be
        SBUF); output flushes.
  Pool: tiny end-chain subtract only (Pool tensor ops measure ~2.2ns/elem +
        ~550ns/instruction — too slow for bulk work, and cannot touch PSUM).

End-chain (overlaps the DVE tail): copy the [32, 1024] denominator tile to
SBUF, 4 small PE transposes back to token-major, then weights =
exp(v - ln(D)) — Ln+Exp on ACT, so no reciprocal or multiply ever touches
the DVE queue. Weights flush in one DMA; indices flush in 4 rolling DMAs.
"""

import numpy as np

TOKENS = 131072
EXPERTS = 256
K = 8
N_CORES = 8
TOK_PER_CORE = TOKENS // N_CORES  # 16384
P = 128
TT = TOK_PER_CORE // P  # 128 token rows per partition
HC = 4  # steady-state subtiles per chunk
NCHUNK = 32  # steady-state chunk count (TT / HC)

# Short prologue so the first DMA lands fast and DVE spins up early. The
# first 6 chunks together cover exactly 2 steady-state chunks (8 subtiles),
# and are folded into denominator rows 0 and 1.
CHUNKS = [1, 1, 1, 1, 2, 2] + [4] * 30
assert sum(CHUNKS) == TT

# Rolling index-output flushes (weights flush once at the end).
IFLUSH_AT = (32, 64, 96, TT)

_PROGRAM_CACHE = {}


def _build_program():
    import concourse.tile as tile
    from concourse import bacc, masks, mybir

    f32 = mybir.dt.float32
    bf16 = mybir.dt.bfloat16
    u32 = mybir.dt.uint32
    Exp = mybir.ActivationFunctionType.Exp
    Ln = mybir.ActivationFunctionType.Ln

    nc = bacc.Bacc("TRN2", debug=False, num_devices=N_CORES)

    g_dram = nc.dram_tensor(
        "gating", [TOK_PER_CORE, EXPERTS], f32, kind="ExternalInput"
    ).ap()
    w_dram = nc.dram_tensor(
        "weights", [TOK_PER_CORE, K], f32, kind="ExternalOutput"
    ).ap()
    i_dram = nc.dram_tensor(
        "indices", [TOK_PER_CORE, K], u32, kind="ExternalOutput"
    ).ap()

    # token = p*TT + tt: partition-major views
    g_v = g_dram.rearrange("(p tt) e -> p tt e", p=P)  # [128, 128, 256]
    w_v = w_dram.rearrange("(p tt) k -> p tt k", p=P)  # [128, 128, 8]
    i_v = i_dram.rearrange("(p tt) k -> p tt k", p=P)

    with tile.TileContext(nc) as tc:
        with (
            tc.tile_pool(name="gin", bufs=6) as gin_pool,
            tc.tile_pool(name="expbuf", bufs=3) as exp_pool,
            tc.tile_pool(name="outs", bufs=3) as out_pool,
            tc.tile_pool(name="persist", bufs=1) as persist_pool,
            tc.tile_pool(name="psum", bufs=2, space="PSUM") as psum_pool,
            tc.tile_pool(name="psump", bufs=1, space="PSUM") as psump_pool,
        ):
            # persistent per-core result buffers
            vbuf = persist_pool.tile([P, TT, K], f32, name="vbuf")
            ibuf = persist_pool.tile([P, TT, K], u32, name="ibuf")
            wbuf = persist_pool.tile([P, TT, K], f32, name="wbuf")

            # constants
            zbias = persist_pool.tile([P, 1], f32, name="zbias")
            nc.gpsimd.memset(zbias, 0.0)
            ident = persist_pool.tile([P, P], bf16, name="ident")
            masks.make_identity(nc, ident[:])
            oneh = persist_pool.tile([P, 2 * NCHUNK], bf16, name="oneh")
            nc.gpsimd.memset(oneh, 0.0)
            nc.gpsimd.memset(oneh[:, NCHUNK - 1 : NCHUNK], 1.0)

            # persistent PSUM accumulator: dT[c, (t, tok)] = per-token
            # denominator for subtile 4c+t, token-partition tok
            dT = psump_pool.tile([NCHUNK, HC * 2 * P], f32, name="dT")

            pend_mm = []  # deferred (quad_tiles, quad_index)

            def emit_mms(flat, q, tn, off, last):
                # accumulate tn subtiles of quad q (starting at subtile
                # offset `off` within the quad) into dT row q; rhs free is
                # capped at 512, so split into 512-wide pieces
                base = off * 2 * P
                n = tn * 2 * P
                for piece in range(0, n, 512):
                    w = min(512, n - piece)
                    # start=True zeroes the target's whole PSUM bank, so it
                    # may only be set on the FIRST write to each 512-f32 bank
                    nc.tensor.matmul(
                        dT[:, base + piece : base + piece + w],
                        oneh[:, NCHUNK - 1 - q : 2 * NCHUNK - 1 - q],
                        flat[:, piece : piece + w],
                        start=(q == 0 and (base + piece) % 512 == 0),
                        stop=(last and piece + w == n),
                        skip_group_check=True,
                    )

            ct = 0
            ifi = 0
            qi = 0  # current quad (group of 4 subtiles = one dT row)
            qfill = 0  # subtiles accumulated into the current quad
            qtiles = []  # (etT tile, T) pieces of the current quad
            for ci, T in enumerate(CHUNKS):
                gt = gin_pool.tile([P, T * EXPERTS], f32, name=f"gt{ci}", tag="gt")
                nc.sync.dma_start(out=gt, in_=g_v[:, ct : ct + T, :])
                gt3 = gt.rearrange("p (t e) -> p t e", t=T)

                for t in range(T):
                    nc.vector.max(out=vbuf[:, ct + t, :], in_=gt3[:, t, :])
                for t in range(T):
                    nc.vector.max_index(
                        out=ibuf[:, ct + t, :],
                        in_max=vbuf[:, ct + t, :],
                        in_values=gt3[:, t, :],
                    )

                # ACT: fused Exp of the chunk -> bf16 (denominator terms)
                etg = exp_pool.tile([P, T, EXPERTS], bf16, name=f"etg{ci}", tag="etg")
                nc.scalar.activation(out=etg, in_=gt3, func=Exp, bias=zbias)

                # PE: transpose exp into PSUM; DMA it back to SBUF for the
                # denominator matmul (matmul rhs must be SBUF)
                etTp = psum_pool.tile([P, T, 2, P], bf16, name=f"etTp{ci}", tag="etTp")
                for t in range(T):
                    for h in range(2):
                        nc.tensor.transpose(
                            etTp[:, t, h, :],
                            etg[:, t, h * P : (h + 1) * P],
                            ident,
                        )
                etT = exp_pool.tile([P, T, 2, P], bf16, name=f"etT{ci}", tag="etT")
                nc.scalar.copy(out=etT, in_=etTp[:, :T, :, :])
                qtiles.append((etT, T))
                qfill += T
                if qfill == HC:
                    pend_mm.append((qtiles, qi))
                    qtiles = []
                    qfill = 0
                    qi += 1
                assert qfill < HC

                # PE: denominator matmuls deferred ~2 quads so the PE queue
                # never waits on the exp/transpose/DMA round-trip mid-stream
                while len(pend_mm) > 2:
                    qt, q = pend_mm.pop(0)
                    off = 0
                    for etT_, tn in qt:
                        emit_mms(
                            etT_.rearrange("p t h x -> p (t h x)"),
                            q,
                            tn,
                            off,
                            last=False,
                        )
                        off += tn

                ct += T
                if ifi < len(IFLUSH_AT) and ct >= IFLUSH_AT[ifi]:
                    lo = IFLUSH_AT[ifi - 1] if ifi else 0
                    nc.sync.dma_start(out=i_v[:, lo:ct, :], in_=ibuf[:, lo:ct, :])
                    ifi += 1

            while pend_mm:
                qt, q = pend_mm.pop(0)
                off = 0
                for j, (etT_, tn) in enumerate(qt):
                    emit_mms(
                        etT_.rearrange("p t h x -> p (t h x)"),
                        q,
                        tn,
                        off,
                        last=(not pend_mm and j == len(qt) - 1),
                    )
                    off += tn

            # ---- end-chain: denominators -> weights (overlaps DVE tail) ----
            dTs = out_pool.tile([NCHUNK, HC * 2 * P], f32, name="dTs")
            nc.scalar.copy(out=dTs, in_=dT)
            # dTs[c, (t, h, tok)] -> D[tok, (c, t, h)]; fold the two halves
            # with a strided self-add first: D2[c, (t, tok)] = h0 + h1
            d2 = out_pool.tile([NCHUNK, HC, P], f32, name="d2")
            dTs4 = dTs.rearrange("c (t h x) -> c t h x", t=HC, h=2)
            nc.gpsimd.tensor_tensor(
                out=d2,
                in0=dTs4[:, :, 0, :],
                in1=dTs4[:, :, 1, :],
                op=mybir.AluOpType.add,
            )
            dback = psum_pool.tile([P, HC, NCHUNK], f32, name="dback", tag="etTp")
            idf = persist_pool.tile([P, P], f32, name="identf")
            masks.make_identity(nc, idf[:])
            for t in range(HC):
                nc.tensor.transpose(dback[:, t, :], d2[:, t, :], idf[:NCHUNK, :NCHUNK])
            # token tt = 4c + t -> iterate (c, t)
            lnd = out_pool.tile([P, TT], f32, name="lnd")
            nc.scalar.activation(
                out=lnd.rearrange("p (c t) -> p c t", t=HC),
                in_=dback.rearrange("p t c -> p c t"),
                func=Ln,
                bias=zbias,
            )
            evv = out_pool.tile([P, TT, K], f32, name="evv")
            nc.gpsimd.tensor_tensor(
                out=evv,
                in0=vbuf,
                in1=lnd.rearrange("p (t one) -> p t one", one=1).to_broadcast(
                    [P, TT, K]
                ),
                op=mybir.AluOpType.subtract,
            )
            nc.scalar.activation(out=wbuf, in_=evv, func=Exp, bias=zbias)
            nc.sync.dma_start(out=w_v, in_=wbuf)

    nc.compile()
    return nc


def kernel(**inputs) -> tuple:
    from concourse.bass_utils import run_bass_kernel_spmd

    gating = np.ascontiguousarray(np.asarray(inputs["gating_output"], dtype=np.float32))
    topk = int(np.asarray(inputs.get("topk", K)))
    assert topk == K, f"kernel hardcodes top-{K}, got topk={topk}"
    assert gating.shape == (TOKENS, EXPERTS), gating.shape

    if "nc" not in _PROGRAM_CACHE:
        _PROGRAM_CACHE["nc"] = _build_program()
    nc = _PROGRAM_CACHE["nc"]

    shards = gating.reshape(N_CORES, TOK_PER_CORE, EXPERTS)
    in_maps = [{"gating": shards[c]} for c in range(N_CORES)]
    res = run_bass_kernel_spmd(nc, in_maps, core_ids=list(range(N_CORES)))
    _PROGRAM_CACHE["last_results"] = res

    weights = np.concatenate([r["weights"] for r in res.results], axis=0)
    indices = np.concatenate([r["indices"] for r in res.results], axis=0)
    return weights.astype(np.float32, copy=False), indices.astype(np.int32, copy=False)


# revision 23
# speedup vs baseline: 1.1787x; 1.0035x over previous
"""MoE routing kernel for Trainium2: softmax over 256 experts + top-8 per token.

Full input: gating_output [131072, 256] f32. Output: (topk_weights f32,
topk_indices int32), both [131072, 8] — matching jax.lax.top_k semantics
(values descending, ties broken by lowest index first).

Strategy: shard tokens row-wise across 8 NeuronCores (16384 tokens each; the
computation is row-local so no communication). Per core, token = p*128 + tt
(partition-major): partition p owns 128 consecutive tokens, processed in
chunks of T token rows per partition (T<=4), so each chunk's input DMA is
128 descriptors of T KiB contiguous.

Engine split — DVE is the bottleneck and runs NOTHING but the top-k:
  DVE : per subtile [128, 256]: InstMax (top-8 raw logits, descending) then
        InstMaxIndex (indices; duplicates get ascending distinct indices —
        matches jax.lax.top_k tie rules). Two full 256-element scans per
        subtile is the ISA floor: the input has exact-duplicate and <1.5e-5
        near-tie rows at the top-8 boundary, so no approximate or compressed
        selection scheme is exact. ~723ns/subtile * 128 subtiles ~= 93us,
        and the measured stream runs at ~727ns/subtile back-to-back.
  ACT : ONE fused Exp per chunk (f32 SBUF -> bf16 SBUF) plus a PSUM->SBUF
        copy of the transposed exp. The bf16 rounding only touches
        denominator terms (~0.1% on the sum, vs 2e-2 tol); top-8 weights
        are recomputed in f32 at the end. Max-subtraction is skipped:
        |x| <= 5.5 keeps exp well inside f32 range; softmax is
        shift-invariant.
  PE  : per subtile, two bf16 transposes of exp into PSUM ([tok,e]->[e,tok]),
        then per quad-of-4-subtiles TWO ones-matmuls (sliding one-hot
        stationary) that accumulate per-token exp-sums into [16, 1024] PSUM
        accumulators — row q holds quad q's 512 token denominators. The
        denominators thus cost the otherwise-idle tensor engine ~2.5
        instructions/subtile.
  Pool: small end-chain tensor ops only (Pool measures ~2.2ns/elem +
        ~550ns/instruction and cannot access PSUM — unusable for bulk).

The weights path runs twice (once per 16-quad half, each with its own PSUM
accumulator): copy accumulator to SBUF, fold the two expert-half sums, 4
small PE transposes back to token-major, then weights = exp(v - ln(D)) —
Ln+Exp on ACT, so no reciprocal or multiply ever touches the DVE queue.
Half A completes mid-stream; only half B (gated by the last max8) remains
at the end, minimizing the serial tail. Constants are emitted after the
first two chunks so the input-DMA stream starts as early as possible.
"""

import numpy as np

TOKENS = 131072
EXPERTS = 256
K = 8
N_CORES = 8
TOK_PER_CORE = TOKENS // N_CORES  # 16384
P = 128
TT = TOK_PER_CORE // P  # 128 token rows per partition
HC = 4  # subtiles per quad (denominator-row granularity)
NQ = 32  # quads
NQH = 16  # quads per half

# Short prologue so the first DMA lands fast and DVE spins up early. The
# first 2 chunks together cover one quad.
CHUNKS = [1, 3] + [4] * 31
assert sum(CHUNKS) == TT

# Rolling index-output flushes.
IFLUSH_AT = (32, 64, 96, TT)

_PROGRAM_CACHE = {}


def _build_program():
    import concourse.tile as tile
    from concourse import bacc, masks, mybir

    f32 = mybir.dt.float32
    bf16 = mybir.dt.bfloat16
    u32 = mybir.dt.uint32
    Exp = mybir.ActivationFunctionType.Exp
    Ln = mybir.ActivationFunctionType.Ln

    nc = bacc.Bacc("TRN2", debug=False, num_devices=N_CORES)

    g_dram = nc.dram_tensor(
        "gating", [TOK_PER_CORE, EXPERTS], f32, kind="ExternalInput"
    ).ap()
    w_dram = nc.dram_tensor(
        "weights", [TOK_PER_CORE, K], f32, kind="ExternalOutput"
    ).ap()
    i_dram = nc.dram_tensor(
        "indices", [TOK_PER_CORE, K], u32, kind="ExternalOutput"
    ).ap()

    # token = p*TT + tt: partition-major views
    g_v = g_dram.rearrange("(p tt) e -> p tt e", p=P)  # [128, 128, 256]
    w_v = w_dram.rearrange("(p tt) k -> p tt k", p=P)  # [128, 128, 8]
    i_v = i_dram.rearrange("(p tt) k -> p tt k", p=P)

    with tile.TileContext(nc) as tc:
        with (
            tc.tile_pool(name="gin", bufs=6) as gin_pool,
            tc.tile_pool(name="expbuf", bufs=3) as exp_pool,
            tc.tile_pool(name="outs", bufs=3) as out_pool,
            tc.tile_pool(name="persist", bufs=1) as persist_pool,
            tc.tile_pool(name="psum", bufs=2, space="PSUM") as psum_pool,
            tc.tile_pool(name="psump", bufs=1, space="PSUM") as psump_pool,
        ):
            # persistent per-core result buffers
            vbuf = persist_pool.tile([P, TT, K], f32, name="vbuf")
            ibuf = persist_pool.tile([P, TT, K], u32, name="ibuf")
            wbuf = persist_pool.tile([P, TT, K], f32, name="wbuf")

            # constants (filled by emit_consts, deferred past the first
            # chunks so the input-DMA stream starts immediately)
            zbias = persist_pool.tile([P, 1], f32, name="zbias")
            ident = persist_pool.tile([P, P], bf16, name="ident")
            oneh = persist_pool.tile([P, 2 * NQ], bf16, name="oneh")

            def emit_consts():
                nc.gpsimd.memset(zbias, 0.0)
                masks.make_identity(nc, ident[:])
                nc.gpsimd.memset(oneh, 0.0)
                nc.gpsimd.memset(oneh[:, NQ - 1 : NQ], 1.0)

            # per-half PSUM denominator accumulators:
            # dt_half[q%16, (t, h, tok)] for quad q
            dthalf = [
                psump_pool.tile([NQH, HC * 2 * P], f32, name=f"dth{i}")
                for i in range(2)
            ]

            def emit_mms(flat, q, tn, off, last):
                # accumulate tn subtiles of quad q (subtile offset `off`
                # inside the quad) into row q%16 of its half's accumulator;
                # rhs free is capped at 512 -> 512-wide pieces
                dt = dthalf[q // NQH]
                qq = q % NQH
                base = off * 2 * P
                n = tn * 2 * P
                # split at absolute 512-f32 PSUM bank boundaries: start=True
                # zeroes the target's whole bank, so it may only be set on
                # the bank-aligned first write of quad 0
                lo = base
                while lo < base + n:
                    hi = min(base + n, (lo // 512 + 1) * 512)
                    nc.tensor.matmul(
                        dt[:, lo:hi],
                        oneh[:, NQ - 1 - qq : NQ - 1 - qq + NQH],
                        flat[:, lo - base : hi - base],
                        start=(qq == 0 and lo % 512 == 0),
                        stop=(last and hi == base + n),
                        skip_group_check=True,
                    )
                    lo = hi

            def emit_wchain(half):
                # weights for tokens tt in [half*64, half*64+64):
                # D -> ln(D) -> w = exp(v - lnD) -> flush
                t0 = half * NQH * HC
                dts = out_pool.tile([NQH, HC * 2 * P], f32, name=f"dts{half}")
                nc.scalar.copy(out=dts, in_=dthalf[half])
                dts4 = dts.rearrange("c (t h x) -> c t h x", t=HC, h=2)
                d2 = out_pool.tile([NQH, HC, P], bf16, name=f"d2{half}")
                nc.gpsimd.tensor_tensor(
                    out=d2,
                    in0=dts4[:, :, 0, :],
                    in1=dts4[:, :, 1, :],
                    op=mybir.AluOpType.add,
                )
                dback = psum_pool.tile([P, HC, NQH], bf16, name=f"db{half}", tag="db")
                for t in range(HC):
                    nc.tensor.transpose(
                        dback[:, t, :], d2[:, t, :], ident[:NQH, :NQH]
                    )
                lnd = out_pool.tile([P, NQH * HC], f32, name=f"lnd{half}")
                nc.scalar.activation(
                    out=lnd.rearrange("p (c t) -> p c t", t=HC),
                    in_=dback.rearrange("p t c -> p c t"),
                    func=Ln,
                    bias=zbias,
                )
                evv = out_pool.tile([P, NQH * HC, K], f32, name=f"evv{half}")
                nc.gpsimd.tensor_tensor(
                    out=evv,
                    in0=vbuf[:, t0 : t0 + NQH * HC, :],
                    in1=lnd.rearrange("p (t one) -> p t one", one=1).to_broadcast(
                        [P, NQH * HC, K]
                    ),
                    op=mybir.AluOpType.subtract,
                )
                nc.scalar.activation(
                    out=wbuf[:, t0 : t0 + NQH * HC, :], in_=evv, func=Exp, bias=zbias
                )
                nc.sync.dma_start(
                    out=w_v[:, t0 : t0 + NQH * HC, :],
                    in_=wbuf[:, t0 : t0 + NQH * HC, :],
                )

            pend_mm = []  # deferred (quad_tiles, quad_index)
            emitted_q = 0
            a_done = False

            def drain_mm(keep):
                nonlocal emitted_q, a_done
                while len(pend_mm) > keep:
                    qt, q = pend_mm.pop(0)
                    off = 0
                    for j, (etT_, tn) in enumerate(qt):
                        emit_mms(
                            etT_.rearrange("p t h x -> p (t h x)"),
                            q,
                            tn,
                            off,
                            last=(
                                q % NQH == NQH - 1
                                and off + tn == HC
                            ),
                        )
                        off += tn
                    emitted_q = q + 1
                if emitted_q >= NQH and not a_done:
                    emit_wchain(0)
                    a_done = True

            ct = 0
            ifi = 0
            qi = 0
            qfill = 0
            qtiles = []
            pend_p2 = []
            for ci, T in enumerate(CHUNKS):
                gt = gin_pool.tile([P, T * EXPERTS], f32, name=f"gt{ci}", tag="gt")
                nc.sync.dma_start(out=gt, in_=g_v[:, ct : ct + T, :])
                gt3 = gt.rearrange("p (t e) -> p t e", t=T)

                for t in range(T):
                    nc.vector.max(out=vbuf[:, ct + t, :], in_=gt3[:, t, :])
                for t in range(T):
                    nc.vector.max_index(
                        out=ibuf[:, ct + t, :],
                        in_max=vbuf[:, ct + t, :],
                        in_values=gt3[:, t, :],
                    )

                def part2(gt3=gt3, T=T, ci=ci):
                    nonlocal qi, qfill, qtiles
                    # ACT: fused Exp of the chunk -> bf16 (denominator terms)
                    etg = exp_pool.tile(
                        [P, T, EXPERTS], bf16, name=f"etg{ci}", tag="etg"
                    )
                    nc.scalar.activation(out=etg, in_=gt3, func=Exp, bias=zbias)
                    # PE: transpose exp into PSUM; ACT copies it back to
                    # SBUF (matmul rhs must be SBUF; Pool/DMA can't do it)
                    etTp = psum_pool.tile(
                        [P, HC, 2, P], bf16, name=f"etTp{ci}", tag="etTp"
                    )
                    for t in range(T):
                        for h in range(2):
                            nc.tensor.transpose(
                                etTp[:, t, h, :],
                                etg[:, t, h * P : (h + 1) * P],
                                ident,
                            )
                    etT = exp_pool.tile([P, T, 2, P], bf16, name=f"etT{ci}", tag="etT")
                    nc.scalar.copy(out=etT, in_=etTp[:, :T, :, :])
                    qtiles.append((etT, T))
                    qfill += T
                    if qfill == HC:
                        pend_mm.append((qtiles, qi))
                        qtiles = []
                        qfill = 0
                        qi += 1

                if ci < 2:
                    pend_p2.append(part2)
                    if ci == 1:
                        emit_consts()
                        for f in pend_p2:
                            f()
                        pend_p2 = []
                else:
                    part2()

                # PE: denominator matmuls deferred 2 quads so the PE queue
                # never waits on the exp/transpose/copy round-trip
                drain_mm(keep=2)

                ct += T
                if ifi < len(IFLUSH_AT) and ct >= IFLUSH_AT[ifi]:
                    lo = IFLUSH_AT[ifi - 1] if ifi else 0
                    nc.sync.dma_start(out=i_v[:, lo:ct, :], in_=ibuf[:, lo:ct, :])
                    ifi += 1

            drain_mm(keep=0)
            emit_wchain(1)

    nc.compile()
    return nc


def kernel(**inputs) -> tuple:
    from concourse.bass_utils import run_bass_kernel_spmd

    gating = np.ascontiguousarray(np.asarray(inputs["gating_output"], dtype=np.float32))
    topk = int(np.asarray(inputs.get("topk", K)))
    assert topk == K, f"kernel hardcodes top-{K}, got topk={topk}"
    assert gating.shape == (TOKENS, EXPERTS), gating.shape

    if "nc" not in _PROGRAM_CACHE:
        _PROGRAM_CACHE["nc"] = _build_program()
    nc = _PROGRAM_CACHE["nc"]

    shards = gating.reshape(N_CORES, TOK_PER_CORE, EXPERTS)
    in_maps = [{"gating": shards[c]} for c in range(N_CORES)]
    res = run_bass_kernel_spmd(nc, in_maps, core_ids=list(range(N_CORES)))
    _PROGRAM_CACHE["last_results"] = res

    weights = np.concatenate([r["weights"] for r in res.results], axis=0)
    indices = np.concatenate([r["indices"] for r in res.results], axis=0)
    return weights.astype(np.float32, copy=False), indices.astype(np.int32, copy=False)
